# revision 46
# baseline (speedup 1.0000x reference)
"""Quantum multi-head attention TRN2 kernel (self-contained), v2.

Problem: x(4,2048,1024); qp=cos(x+theta) per-head(16x64); q/k/v = qp@W*+b*
(per-head shared 64x64 weights); full softmax attention; merge heads; @Wo+bo.

Sharding v2: 8 cores = (batch b, head-half m).  Each core handles 8 heads
(4 head-pairs) over the full 2048x2048 attention, computes a PARTIAL
out-projection over its 512 embedding dims; host sums the two partials per
batch.  No collectives.

Device algorithm per core:
  ramp:  qpT  = cos(xt_s*2pi + th)   4x[128,2048] bf16 (transposed layout)
         kT/qT = W @ qpT             per local pair (row-tiled PE pairs)
         qpn  = cos in natural layout [qp|1] per head  16x[128, 4*130] bf16
         wvo  = blockdiag(Wv) @ Wo_local (bf16), bvWo+bo broadcast
  attn:  for it(4 query slices) x t(4 pairs): 16 key-chunk units:
         scoresT(j,i) = kT^T q  into TWO independent PSUM tiles
         ([128,2048] pair + [128,1024] single) so exp reads never
         false-serialize against score fills; 2 heads PE-row-tiled
         e = exp(scores/8)  ACT [128,2048]/[128,1024], bf16 out
         ctxT+denom accumulated in PSUM via [qp|1]^T @ e, ctx lagged
         2 cycles behind scores (e-tile queue decouples PE from ACT)
         normalization flushed at stretch end (spread reciprocal via
         DRAM bounce), bf16 ctxT; dummy LDWEIGHTS as PE busy-filler
         keep the HAM clock-gate at K=8/8
         phase4 spread one query-chunk per stretch (prev it-slice);
         host sums the two half-E partials per batch
"""
import numpy as np
import ml_dtypes

import concourse.bass as bass
import concourse.mybir as mybir
import concourse.tile as tile
from concourse.bass_utils import run_bass_kernel_spmd

F32 = mybir.dt.float32
F32R = mybir.dt.float32r
BF16 = mybir.dt.bfloat16
nbf16 = ml_dtypes.bfloat16
PI = float(np.pi)
MAGIC = 12582912.0  # 1.5 * 2**23 round-to-nearest magic
A = mybir.AluOpType
AF = mybir.ActivationFunctionType

B, S, E = 4, 2048, 1024
H, HD = 16, 64
EL = 512           # local embed dims per core (8 heads)
NP = 4             # local head pairs
NCH = 16           # key chunks of 128
NIT = 4            # query slices of 512
N_CORES = 8
TRACE = False
LAST_RES = None


def _split_multiwaits(nc):
    """This container's walrus supports ONE sync-wait per instruction; split
    extras onto single-wait no-ops on the same engine (program order keeps
    semantics)."""
    counter = 0
    for f in nc.m.functions:
        for bb in f.blocks:
            new_insts = []
            for inst in bb.instructions:
                si = inst.sync_info
                if si is not None and si.on_wait and len(si.on_wait) > 1:
                    waits = list(si.on_wait)
                    si.on_wait = [waits[-1]]
                    for w in waits[:-1]:
                        counter += 1
                        new_insts.append(mybir.InstNoOp(
                            name=f"splitw-{counter}",
                            engine=inst.engine,
                            sync_info=mybir.SyncInfo(on_wait=[w], on_update=[]),
                            bass_nofuse=True,
                        ))
                new_insts.append(inst)
            bb.instructions[:] = new_insts
    return counter


def _build():
    nc = bass.Bass("TRN2", target_bir_lowering=False, debug=False)

    # xt_s/xn_s are pre-scaled by 1/(2*pi) on the host
    xt = nc.dram_tensor("xt", [EL, S], F32, kind="ExternalInput")
    xn = nc.dram_tensor("xn", [S, EL], F32, kind="ExternalInput")
    tht = nc.dram_tensor("tht", [128, 1], F32, kind="ExternalInput")
    thbc = nc.dram_tensor("thbc", [128, EL], F32, kind="ExternalInput")
    wq2 = nc.dram_tensor("wq2", [128, HD], BF16, kind="ExternalInput")
    wk2 = nc.dram_tensor("wk2", [128, HD], BF16, kind="ExternalInput")
    wvt2 = nc.dram_tensor("wvt2", [128, HD], F32, kind="ExternalInput")
    wo = nc.dram_tensor("wo", [EL, E], F32, kind="ExternalInput")
    bq2 = nc.dram_tensor("bq2", [128, 1], F32, kind="ExternalInput")
    bk2 = nc.dram_tensor("bk2", [128, 1], F32, kind="ExternalInput")
    bv2 = nc.dram_tensor("bv2", [128, 1], F32, kind="ExternalInput")
    bo_r = nc.dram_tensor("bo_r", [1, E], F32, kind="ExternalInput")
    out = nc.dram_tensor("out", [S, E], F32, kind="ExternalOutput")

    with tile.TileContext(nc) as tc:
        with (
            tc.tile_pool(name="persist", bufs=1) as pp,
            tc.tile_pool(name="qpTp", bufs=2) as qpT_pool,
            tc.tile_pool(name="xtw", bufs=2) as xtw,
            tc.tile_pool(name="rtw", bufs=2) as rtw,
            tc.tile_pool(name="xnw", bufs=2) as xnw,
            tc.tile_pool(name="rnw", bufs=2) as rnw,
        ):
            # ---- persistent consts
            tht_t = pp.tile([128, 1], F32, name="tht_t")
            nc.sync.dma_start(tht_t[:], tht.ap())
            thbc_t = pp.tile([128, EL], F32, name="thbc_t")
            nc.sync.dma_start(thbc_t[:], thbc.ap())
            wq2_t = pp.tile([128, HD], BF16, name="wq2_t")
            nc.sync.dma_start(wq2_t[:], wq2.ap())
            wk2_t = pp.tile([128, HD], BF16, name="wk2_t")
            nc.sync.dma_start(wk2_t[:], wk2.ap())
            wvt2_f = pp.tile([128, HD], F32, name="wvt2_f")
            nc.sync.dma_start(wvt2_f[:], wvt2.ap())
            wvt2_t = pp.tile([128, HD], F32R, name="wvt2_t")
            nc.vector.tensor_copy(wvt2_t[:], wvt2_f[:])
            bq2_t = pp.tile([128, 1], F32, name="bq2_t")
            nc.sync.dma_start(bq2_t[:], bq2.ap())
            bk2_t = pp.tile([128, 1], F32, name="bk2_t")
            nc.sync.dma_start(bk2_t[:], bk2.ap())
            bv2_f = pp.tile([128, 1], F32, name="bv2_f")
            nc.sync.dma_start(bv2_f[:], bv2.ap())
            bv2_t = pp.tile([128, 1], F32R, name="bv2_t")
            nc.vector.tensor_copy(bv2_t[:], bv2_f[:])
            borow_t = pp.tile([1, E], F32, name="borow_t")
            nc.sync.dma_start(borow_t[:], bo_r.ap())
            bobc_t = pp.tile([128, E], F32, name="bobc_t")

            # persistent big arrays
            kT = [pp.tile([128, S], BF16, name=f"kT_{t}") for t in range(NP)]
            qT = [pp.tile([128, S], BF16, name=f"qT_{t}") for t in range(NP)]
            qpn = [pp.tile([128, NP * 130], BF16, name=f"qpn_{j}")
                   for j in range(NCH)]
            ctxT = [pp.tile([128, S], BF16, name=f"ctxT_{t}")
                    for t in range(NP)]
            wvo = [pp.tile([128, E], BF16, name=f"wvo_{t}") for t in range(NP)]

            # ====================== ramp: cos chains ======================
            # ACT order: Sin(qpT0), Sin(qpn 0..15), Sin(qpT 1..3), then Exp
            qpT_tiles = {}

            def cos_qpT(t):
                xt_t = xtw.tile([128, S], F32, name=f"xt_{t}", tag="xt_in")
                nc.sync.dma_start(xt_t[:], xt.ap()[128 * t:128 * t + 128, :])
                nc.vector.tensor_scalar(xt_t[:], xt_t[:], tht_t[:, 0:1], None,
                                        A.add)
                rt = rtw.tile([128, S], F32, name=f"rt_{t}", tag="rt")
                nc.vector.tensor_scalar(rt[:], xt_t[:], MAGIC, MAGIC,
                                        A.add, A.subtract)
                nc.vector.tensor_tensor(xt_t[:], xt_t[:], rt[:], A.subtract)
                qpt = qpT_pool.tile([128, S], BF16, name=f"qpT_{t}", tag="qpT")
                nc.scalar.activation(qpt[:], xt_t[:], AF.Sin,
                                     bias=0.0, scale=2.0 * PI)
                qpT_tiles[t] = qpt

            def cos_qpn(jn):
                xn_t = xnw.tile([128, EL], F32, name=f"xn_{jn}", tag="xn_in")
                nc.sync.dma_start(xn_t[:],
                                  xn.ap()[128 * jn:128 * jn + 128, :])
                nc.vector.tensor_tensor(xn_t[:], xn_t[:], thbc_t[:], A.add)
                tn = rnw.tile([128, EL], F32, name=f"tn_{jn}", tag="rn")
                nc.vector.tensor_scalar(tn[:], xn_t[:], MAGIC, MAGIC,
                                        A.add, A.subtract)
                nc.vector.tensor_tensor(xn_t[:], xn_t[:], tn[:], A.subtract)
                qv = qpn[jn][:].rearrange("p (pr hh c) -> p pr hh c",
                                          pr=NP, c=65)
                nc.vector.memset(qv[:, :, :, 64:65], 1.0)
                uv = xn_t[:].rearrange("p (pr hh d) -> p pr hh d",
                                       pr=NP, d=64)
                nc.scalar.activation(qv[:, :, :, 0:64], uv, AF.Sin,
                                     bias=0.0, scale=2.0 * PI)

            def proj(t, prj_ps):
                # kT/qT for pair t: 4 key chunks x 2 row-tiled MMs each
                qpt = qpT_tiles.pop(t)
                for dst, w_t, b_t in ((kT[t], wk2_t, bk2_t),
                                      (qT[t], wq2_t, bq2_t)):
                    for half in range(2):
                        hs = slice(1024 * half, 1024 * half + 1024)
                        ps = prj_ps.tile([128, 1024], F32,
                                         name=f"prj_{t}_{dst.name}_{half}",
                                         tag="prj")
                        for st in range(2):
                            ss = slice(512 * st, 512 * st + 512)
                            qs = slice(1024 * half + 512 * st,
                                       1024 * half + 512 * st + 512)
                            nc.tensor.matmul(ps[0:64, ss], w_t[0:64, :],
                                             qpt[0:64, qs],
                                             start=True, stop=True)
                            nc.tensor.matmul(ps[64:128, ss], w_t[64:128, :],
                                             qpt[64:128, qs],
                                             start=True, stop=True)
                        nc.scalar.activation(dst[:, hs], ps[:],
                                                 AF.Identity,
                                                 bias=b_t[:, 0:1])

            with (
                tc.tile_pool(name="prj", bufs=1, space="PSUM") as prj_ps,
                tc.tile_pool(name="ps0a", bufs=1, space="PSUM") as ps0a,
                tc.tile_pool(name="ps0b", bufs=2, space="PSUM") as ps0b,
                tc.tile_pool(name="p0", bufs=2) as p0,
                tc.tile_pool(name="dr0", bufs=1, space="DRAM") as dr0,
            ):
                with tc.high_priority():
                    for jn in range(NCH):
                        cos_qpn(jn)
                    for t in range(NP):
                        cos_qpT(t)
                        proj(t, prj_ps)

                # ---- phase 0: wvo = blockdiag(Wv) @ Wo_local, bvWo+bo ----
                bvwo_ps = ps0a.tile([1, E], F32, name="bvwo_ps", tag="bvwo")
                for t in range(NP):
                    wo_f = p0.tile([128, E], F32, name=f"wof_{t}", tag="wo_inf")
                    nc.sync.dma_start(wo_f[:], wo.ap()[128 * t:128 * t + 128, :])
                    wo_t = p0.tile([128, E], F32R, name=f"wo_{t}", tag="wo_in")
                    nc.vector.tensor_copy(wo_t[:], wo_f[:])
                    for nt in range(2):
                        ns = slice(512 * nt, 512 * nt + 512)
                        wvpsA = ps0b.tile([64, 512], F32,
                                          name=f"wvpsA_{t}_{nt}", tag="wvopsA")
                        nc.tensor.matmul(wvpsA[:], wvt2_t[0:64, :],
                                         wo_t[0:64, ns], start=True, stop=True)
                        wvpsB = ps0b.tile([64, 512], F32,
                                          name=f"wvpsB_{t}_{nt}", tag="wvopsB")
                        nc.tensor.matmul(wvpsB[:], wvt2_t[64:128, :],
                                         wo_t[64:128, ns], start=True, stop=True)
                        nc.scalar.activation(wvo[t][0:64, ns], wvpsA[:],
                                                 AF.Copy)
                        nc.scalar.activation(wvo[t][64:128, ns], wvpsB[:],
                                                 AF.Copy)
                        nc.tensor.matmul(bvwo_ps[0:1, ns], bv2_t[:], wo_t[:, ns],
                                         start=(t == 0), stop=(t == NP - 1))
                bosum_t = p0.tile([1, E], F32, name="bosum", tag="bosum")
                nc.vector.tensor_add(bosum_t[:], bvwo_ps[:], borow_t[:])
                bod = dr0.tile([1, E], F32, name="bod", tag="bod")
                nc.sync.dma_start(bod[:], bosum_t[:])
                nc.sync.dma_start(bobc_t[:], bod[:].broadcast_to([128, E]))

            # ====================== attention ======================
            with (
                tc.tile_pool(name="spa", bufs=1, space="PSUM") as ps_a,
                tc.tile_pool(name="spb", bufs=1, space="PSUM") as ps_b,
                tc.tile_pool(name="ps_c", bufs=2, space="PSUM") as ps_c,
                tc.tile_pool(name="e2", bufs=4) as e2_pool,
                tc.tile_pool(name="e1", bufs=4) as e1_pool,
                tc.tile_pool(name="crw", bufs=4) as crw_pool,
                tc.tile_pool(name="nrm", bufs=4) as nrm_pool,
                tc.tile_pool(name="drb", bufs=4, space="DRAM") as dr_pool,
                tc.tile_pool(name="otp", bufs=2) as ot_pool,
            ):
                pending = []

                def flush_pending():
                    for (tt, it_, head, craw) in pending:
                        isl_ = slice(512 * it_, 512 * it_ + 512)
                        sfx = f"{tt}_{it_}_{head}"
                        # denominators -> DRAM -> reload spread over 64
                        # partitions so reciprocal uses 64 lanes, not 1
                        dr1 = dr_pool.tile([1, 512], F32,
                                           name=f"dr1_{sfx}", tag="dr1")
                        nc.sync.dma_start(dr1[:], craw[64:65, :])
                        den8 = nrm_pool.tile([64, 8], F32,
                                             name=f"den8_{sfx}", tag="den8")
                        nc.sync.dma_start(
                            den8[:],
                            dr1[:].rearrange("a (b c) -> (a b) c", c=8))
                        rec8 = nrm_pool.tile([64, 8], F32,
                                             name=f"rec8_{sfx}", tag="rec8")
                        nc.vector.reciprocal(rec8[:], den8[:])
                        dr2 = dr_pool.tile([1, 512], F32,
                                           name=f"dr2_{sfx}", tag="dr2")
                        nc.sync.dma_start(
                            dr2[:].rearrange("a (b c) -> (a b) c", c=8),
                            rec8[:])
                        bc = nrm_pool.tile([64, 512], F32,
                                           name=f"bc_{sfx}", tag="bc")
                        nc.sync.dma_start(bc[:], dr2[:].broadcast_to([64, 512]))
                        rs = slice(64 * head, 64 * head + 64)
                        nc.vector.tensor_mul(ctxT[tt][rs, isl_],
                                             craw[0:64, :], bc[:])
                    pending.clear()

                def phase4_ic(ic):
                    ics = slice(128 * ic, 128 * ic + 128)
                    ot = ot_pool.tile([128, E], F32, name=f"ot_{ic}", tag="ot")
                    for nt in range(2):
                        ns = slice(512 * nt, 512 * nt + 512)
                        ops_ = ps_c.tile([128, 512], F32,
                                         name=f"ops_{ic}_{nt}", tag="ctx")
                        for t in range(NP):
                            nc.tensor.matmul(ops_[:], ctxT[t][:, ics],
                                             wvo[t][:, ns],
                                             start=(t == 0),
                                             stop=(t == NP - 1))
                        nc.vector.tensor_add(ot[:, ns], ops_[:],
                                             bobc_t[:, ns])
                    nc.sync.dma_start(out.ap()[ics, :], ot[:])

                for it in range(NIT):
                    isl = slice(512 * it, 512 * it + 512)
                    for t in range(NP):
                        # phase4 of the previous it-slice is injected into the
                        # first stretch of this one, before cA/cB exist, so
                        # its PE burst overlaps live exp work
                        inject = it > 0 and t == 0
                        cab = []   # lazily allocated ctx accumulators
                        etile = {}  # unit -> (e tile, colA, colB)
                        cur = {}    # current scores tiles

                        def emit_S(u, t=t):
                            pos = u % 3
                            js = slice(128 * u, 128 * u + 128)
                            single = (pos == 2 or u == 15)
                            if pos == 0 and u != 15:
                                cur['a'] = ps_a.tile([128, 2048], F32,
                                                     name=f"sa_{t}_{it}_{u}",
                                                     tag="sa")
                            if single:
                                sp, col = ps_b.tile([128, 1024], F32,
                                                    name=f"sb_{t}_{it}_{u}",
                                                    tag="sb"), 0
                            else:
                                sp, col = cur['a'], 1024 * pos
                            nc.tensor.matmul(sp[:, col:col + 512],
                                             kT[t][0:64, js], qT[t][0:64, isl],
                                             start=True, stop=True)
                            nc.tensor.matmul(sp[:, col + 512:col + 1024],
                                             kT[t][64:128, js],
                                             qT[t][64:128, isl],
                                             start=True, stop=True)
                            # exp groups: (3c,3c+1) -> pair tile; (3c+2) -> b
                            if single:
                                e = e1_pool.tile([128, 1024], BF16,
                                                 name=f"e1_{t}_{it}_{u}",
                                                 tag="e1")
                                nc.scalar.activation(
                                    e[:], sp[:], AF.Exp,
                                    bias=0.0, scale=0.125)
                                etile[u] = (e, 0, 512)
                            elif pos == 1:
                                e = e2_pool.tile([128, 2048], BF16,
                                                 name=f"e2_{t}_{it}_{u}",
                                                 tag="e2")
                                nc.scalar.activation(e[:], cur['a'][:],
                                                     AF.Exp, bias=0.0,
                                                     scale=0.125)
                                etile[u - 1] = (e, 0, 512)
                                etile[u] = (e, 1024, 1536)

                        def emit_ctx(u, t=t):
                            if not cab:
                                cab.append(ps_c.tile([65, 512], F32,
                                                     name=f"cA_{t}_{it}",
                                                     tag="ctx"))
                                cab.append(ps_c.tile([65, 512], F32,
                                                     name=f"cB_{t}_{it}",
                                                     tag="ctx"))
                            e, ca, cb = etile.pop(u)
                            st_, sp_ = (u == 0), (u == 15)
                            nc.tensor.matmul(
                                cab[0][:], qpn[u][:, 130 * t:130 * t + 65],
                                e[:, ca:ca + 512], start=st_, stop=sp_)
                            nc.tensor.matmul(
                                cab[1][:], qpn[u][:, 130 * t + 65:130 * t + 130],
                                e[:, cb:cb + 512], start=st_, stop=sp_)

                        # cycle-ordered: all S of a 3-unit cycle first,
                        # then the 2-cycle-lagged ctx batch; one ic of the
                        # previous it-slice's out-projection per stretch,
                        # placed before the ctx accumulators exist
                        for c in range(6):
                            for u in range(3 * c, min(3 * c + 3, NCH)):
                                emit_S(u)
                            if c == 1 and it > 0 and t > 0:
                                # t=0 would wait on the boundary flush;
                                # spread the 4 chunks over stretches 1-3
                                phase4_ic(4 * (it - 1) + t - 1)
                                if t == 3:
                                    phase4_ic(4 * (it - 1) + 3)
                            if c >= 2:
                                for u in range(3 * (c - 2), 3 * c - 3):
                                    emit_ctx(u)
                            for _ in range(2):
                                nc.tensor.ldweights(
                                    weights=kT[NP - 1 - t][0:128, 0:128])
                        for u in range(12, NCH):
                            emit_ctx(u)

                        for head in range(2):
                            craw = crw_pool.tile(
                                [65, 512], F32,
                                name=f"craw_{t}_{it}_{head}", tag="craw")
                            nc.vector.tensor_copy(craw[:], cab[head][:])
                            pending.append((t, it, head, craw))
                        flush_pending()
                for ic in range(4 * (NIT - 1), 4 * NIT):
                    for _ in range(6):
                        nc.tensor.ldweights(
                            weights=kT[0][0:128, 0:128])
                    phase4_ic(ic)

    return nc


def kernel(x, theta, Wq, bq, Wk, bk, Wv, bv, Wo, bo):
    x = np.asarray(x, np.float32)
    theta = np.asarray(theta, np.float32)
    Wq = np.asarray(Wq, np.float32)
    Wk = np.asarray(Wk, np.float32)
    Wv = np.asarray(Wv, np.float32)
    Wo = np.asarray(Wo, np.float32)
    bq = np.asarray(bq, np.float32)
    bk = np.asarray(bk, np.float32)
    bv = np.asarray(bv, np.float32)
    bo = np.asarray(bo, np.float32)

    nc = _build()
    _split_multiwaits(nc)

    in_maps = make_in_maps(x, theta, Wq, bq, Wk, bk, Wv, bv, Wo, bo)

    kw = {}
    if TRACE:
        kw = dict(trace=True, trace_cores=[0])
    res = run_bass_kernel_spmd(nc, in_maps, core_ids=list(range(N_CORES)), **kw)
    global LAST_RES
    LAST_RES = res

    out = np.empty((B, S, E), np.float32)
    for b in range(B):
        out[b] = res.results[2 * b]["out"] + res.results[2 * b + 1]["out"]
    return out


def make_in_maps(x, theta, Wq, bq, Wk, bk, Wv, bv, Wo, bo):
    inv2pi = 1.0 / (2.0 * PI)
    th2 = np.concatenate([theta, theta]).reshape(128, 1)
    tht = ((th2 + PI / 2) / (2 * PI)).astype(np.float32)
    thbc = np.tile(
        ((np.tile(theta, 8) + PI / 2) / (2 * PI)).astype(np.float32)
        .reshape(1, EL),
        (128, 1),
    )
    wq2 = np.concatenate([Wq, Wq], axis=0).astype(nbf16)
    wk2 = np.concatenate([Wk, Wk], axis=0).astype(nbf16)
    wvt2 = np.ascontiguousarray(
        np.concatenate([Wv.T, Wv.T], axis=0), dtype=np.float32)
    bq2 = np.concatenate([bq, bq]).reshape(128, 1).astype(np.float32)
    bk2 = np.concatenate([bk, bk]).reshape(128, 1).astype(np.float32)
    bv2 = np.concatenate([bv, bv]).reshape(128, 1).astype(np.float32)
    bo_z = np.zeros((1, E), np.float32)

    in_maps = []
    for c in range(N_CORES):
        b, m = c // 2, c % 2
        cols = slice(EL * m, EL * (m + 1))
        xs = x[b] * inv2pi
        in_maps.append(dict(
            xt=np.ascontiguousarray(xs.T[cols, :]),
            xn=np.ascontiguousarray(xs[:, cols]),
            tht=tht, thbc=thbc, wq2=wq2, wk2=wk2, wvt2=wvt2,
            wo=np.ascontiguousarray(Wo[cols, :]),
            bq2=bq2, bk2=bk2, bv2=bv2,
            bo_r=(bo.reshape(1, E).astype(np.float32) if m == 0 else bo_z),
        ))
    return in_maps
